# revision 1
# baseline (speedup 1.0000x reference)
"""Block-causal multi-head self-attention for TRN2, sharded over 8 NeuronCores.

Problem (hardcoded): B=2, T=2048 (512 frames x 4 animals), C=512, H=8 heads,
D=64. Block-causal mask = kron(tril(frames), ones(4,4)); key mask is all-ones
per the input spec (a numpy fallback handles the general case).

Sharding: core i handles batch b = i//4 and heads (2*(i%4), 2*(i%4)+1).
Wq/Wk/Wv are column-sharded (128 cols per core), Wp row-sharded (128 rows per
core). Each core emits a partial [T, C] output projection; the host sums the
4 partials per batch and adds bp.

On-core algorithm (all f32 data; matmuls run in fp32r "transpose mode"):
  xT = transpose(x)  (PE transposes)
  qT/kT/vT = W.T @ xT (+bias via K=1 matmul; softmax scale folded into Wq)
  per head h, per 256-wide query chunk qc:
    S^T[j,i] = kT^T(128-j-block) @ qT(chunk)    (K=64, blocks j<=diag)
    diagonal tri-mask added via a rank-64 indicator matmul (-1e9 offside)
    P^T = exp(S^T)  (ACT, PSUM->SBUF, groups of 4 j-blocks)
    O^T_aug[65,256] += V_aug(block).T @ P^T     (V col 64 = ones -> row 64 = l)
  l column per 128-row t-tile via a [65,1] selector matmul; rl = 1/l
  partial = (O^T_h[:64].T @ Wp_h) * rl_h summed over the 2 heads (ACT/DVE)
"""

import math

import numpy as np

import concourse.bass as bass
import concourse.bacc as bacc
import concourse.tile as tile
from concourse import mybir
from concourse.bass_utils import run_bass_kernel_spmd

B, T, C, H, D = 2, 2048, 512, 8, 64
NF, NA = 512, 4
NCORES = 8
HPC = 2            # heads per core
CPB = 4            # cores per batch
SCALE = 1.0 / math.sqrt(D)
NEG = -1.0e9
IC = 256           # query-chunk width
NQC = T // IC      # 8
GRP = 4            # j-blocks (128 keys each) per exp group

F32 = mybir.dt.float32
F32R = mybir.dt.float32r

USE_F32R = True
PHASE_LIMIT = None   # None | "A" | "B" | "C" | "D" (debug: stop after phase)
DBG_HEADS = None     # debug: restrict head loop
DBG_NOMASK = False   # debug: skip diagonal mask matmuls


def _dump(nc, out_d, t, row0, ncols=512):
    p, fsz = t.shape[0], t.shape[1]
    n = min(fsz // ncols, (T - row0) // 128)
    for i in range(n):
        nc.sync.dma_start(
            out=out_d[row0 + i * 128 : row0 + i * 128 + p, 0:ncols],
            in_=t[:, i * ncols : (i + 1) * ncols].bitcast(F32),
        )


def _emit(ctx, tc, out_d, in_d):
    nc = tc.nc
    WDT = F32R if USE_F32R else F32
    r = lambda ap: ap
    ts = bass.ts
    Exp = mybir.ActivationFunctionType.Exp
    mult, add = mybir.AluOpType.mult, mybir.AluOpType.add

    const = ctx.enter_context(tc.tile_pool(name="const", bufs=1))
    big = ctx.enter_context(tc.tile_pool(name="big", bufs=1))

    ident = const.tile([128, 128], WDT)
    nc.sync.dma_start(out=ident[:], in_=in_d["ident"][:])
    sel = const.tile([65, 1], F32)
    nc.sync.dma_start(out=sel[:], in_=in_d["sel65"][:])
    identh = const.tile([128, 64], WDT)
    nc.sync.dma_start(out=identh[:], in_=in_d["identh"][:])

    w_sb, b_sb = {}, {}
    for nm in ("wq", "wk", "wv"):
        w = const.tile([128, 512], WDT, tag=f"w_{nm}")
        for cb in range(4):
            nc.sync.dma_start(out=w[:, ts(cb, 128)], in_=in_d[nm][ts(cb, 128), :])
        w_sb[nm] = w
        bt = const.tile([1, 128], WDT, tag=f"b_{nm}")
        nc.sync.dma_start(out=bt[:], in_=in_d["b" + nm[1]][:])
        b_sb[nm] = bt
    wps = []
    for h in range(HPC):
        wph = const.tile([64, 512], WDT, tag=f"wp{h}", name=f"wp{h}")
        nc.sync.dma_start(out=wph[:], in_=in_d["wp"][ts(h, 64), :])
        wps.append(wph)

    ones_row = const.tile([1, T], WDT, tag="ones")
    nc.sync.dma_start(out=ones_row[:], in_=in_d["ones_row"][:])
    ones_col = const.tile([128, 1], WDT, tag="ones_col")
    nc.sync.dma_start(out=ones_col[:], in_=in_d["ones_col"][:])

    xT = big.tile([128, 4 * T], WDT, tag="xT")
    qA = [big.tile([128, T], WDT, tag=f"qA{h}", name=f"qA{h}") for h in range(HPC)]
    kA = [big.tile([128, T], WDT, tag=f"kA{h}", name=f"kA{h}") for h in range(HPC)]
    vT = big.tile([128, T], WDT, tag="vT")
    # mask rows: head0 tiles carry them in partitions 64:128, head1 in 0:64
    nc.sync.dma_start(out=kA[0][64:128, :], in_=in_d["mask_k"][:])
    nc.sync.dma_start(out=kA[1][0:64, :], in_=in_d["mask_k"][:])
    nc.sync.dma_start(out=qA[0][64:128, :], in_=in_d["mask_q"][:])
    nc.sync.dma_start(out=qA[1][0:64, :], in_=in_d["mask_q"][:])
    Vb = big.tile([128, HPC * 16 * 65], WDT, tag="Vb")
    OTs = [big.tile([65, T], WDT, tag=f"OT{h}", name=f"OT{h}") for h in range(HPC)]
    rl = big.tile([128, HPC * 16], F32, tag="rl")

    # Phase A: stream x in, transpose to xT[c, t] (4 c-chunks along free dim).
    with tc.tile_pool(name="xin", bufs=4) as xin, \
         tc.tile_pool(name="pst", bufs=4, space="PSUM") as pst:
        for tb in range(16):
            xt_ = xin.tile([128, 512], WDT)
            nc.sync.dma_start(out=xt_[:], in_=in_d["x"][ts(tb, 128), :])
            for cb in range(4):
                pt = pst.tile([128, 128], WDT)
                nc.tensor.transpose(pt[:], xt_[:, ts(cb, 128)], ident[:])
                nc.vector.tensor_copy(
                    xT[:, cb * T + tb * 128 : cb * T + (tb + 1) * 128], pt[:]
                )

    if PHASE_LIMIT == "A":
        _dump(nc, out_d, xT, 0)
        return

    # Phase B: projections -> qT/kT/vT in [head_ch, t] layout.
    with tc.tile_pool(name="psproj", bufs=2, space="PSUM") as psp:
        for nm, dst in (("wq", qA), ("wk", kA), ("wv", vT)):
            for t5 in range(4):
                ps = psp.tile([128, 512], F32)
                for cb in range(4):
                    nc.tensor.matmul(
                        ps[:],
                        r(w_sb[nm][:, ts(cb, 128)]),
                        r(xT[:, cb * T + t5 * 512 : cb * T + (t5 + 1) * 512]),
                        start=(cb == 0),
                        stop=False,
                    )
                nc.tensor.matmul(
                    ps[:], r(b_sb[nm][:]), r(ones_row[:, ts(t5, 512)]),
                    start=False, stop=True,
                )
                if nm == "wv":
                    nc.vector.tensor_copy(dst[:, ts(t5, 512)], ps[:])
                else:
                    nc.vector.tensor_copy(dst[0][0:64, ts(t5, 512)], ps[0:64, :])
                    nc.vector.tensor_copy(dst[1][64:128, ts(t5, 512)], ps[64:128, :])

    if PHASE_LIMIT == "B":
        _dump(nc, out_d, qA[0], 0)
        _dump(nc, out_d, kA[0], 512)
        _dump(nc, out_d, vT, 1024)
        return

    # Phase C: V_aug blocks [128 j, 65] per (head, j-block); col 64 = ones.
    with tc.tile_pool(name="psv", bufs=4, space="PSUM") as psv:
        for h in range(HPC):
            for jb in range(16):
                pv = psv.tile([128, 64], WDT)
                nc.tensor.transpose(
                    pv[:], vT[ts(h, 64), ts(jb, 128)], identh[ts(h, 64), :]
                )
                base = (h * 16 + jb) * 65
                nc.vector.tensor_copy(Vb[:, base : base + 64], pv[:])
                nc.vector.tensor_copy(Vb[:, base + 64 : base + 65], ones_col[:])

    if PHASE_LIMIT == "C":
        _dump(nc, out_d, Vb, 0)
        return

    # Phase D/E: attention + projection, interleaved per query chunk.
    with tc.tile_pool(name="pss", bufs=2, space="PSUM") as pss, \
         tc.tile_pool(name="psot", bufs=1, space="PSUM") as psot, \
         tc.tile_pool(name="psl", bufs=1, space="PSUM") as pslp, \
         tc.tile_pool(name="pspr", bufs=2, space="PSUM") as pspr, \
         tc.tile_pool(name="ptp", bufs=3) as ptp, \
         tc.tile_pool(name="comb", bufs=4) as comb:
        psl = pslp.tile([128, HPC * 16], F32)
        for qc in range(NQC):
            nbj = 2 * (qc + 1)
            for h in range(DBG_HEADS or HPC):
                hs = slice(0, 64) if h == 0 else slice(64, 128)
                ot = psot.tile([65, IC], F32)
                for g0 in range(0, nbj, GRP):
                    ng = min(GRP, nbj - g0)
                    ps = pss.tile([128, GRP * IC], F32)
                    for k in range(ng):
                        jb = g0 + k
                        sl = ps[:, ts(k, IC)]
                        diag = (jb >= 2 * qc) and not DBG_NOMASK
                        if diag:
                            nc.tensor.matmul(
                                sl,
                                r(kA[h][:, ts(jb, 128)]),
                                r(qA[h][:, ts(qc, IC)]),
                                start=True, stop=True,
                            )
                        else:
                            nc.tensor.matmul(
                                sl,
                                r(kA[h][hs, ts(jb, 128)]),
                                r(qA[h][hs, ts(qc, IC)]),
                                start=True, stop=True,
                            )
                    ptt = ptp.tile([128, GRP * IC], WDT)
                    nc.scalar.activation(ptt[:, 0 : ng * IC], ps[:, 0 : ng * IC], Exp)
                    for k in range(ng):
                        jb = g0 + k
                        vbase = (h * 16 + jb) * 65
                        nc.tensor.matmul(
                            ot[:],
                            r(Vb[:, vbase : vbase + 65]),
                            r(ptt[:, ts(k, IC)]),
                            start=(jb == 0),
                            stop=(jb == nbj - 1),
                        )
                nc.vector.tensor_copy(OTs[h][:, ts(qc, IC)], ot[:])
            if PHASE_LIMIT == "D":
                continue
            for t2 in (2 * qc, 2 * qc + 1):
                for h in range(HPC):
                    col = h * 16 + t2
                    nc.tensor.matmul(
                        psl[:, col : col + 1],
                        OTs[h][:, ts(t2, 128)].bitcast(F32),
                        sel[:],
                        start=True,
                        stop=True,
                    )
                    nc.vector.reciprocal(rl[:, col : col + 1], psl[:, col : col + 1])
                pr0 = pspr.tile([128, 512], F32, tag="pr")
                nc.tensor.matmul(
                    pr0[:], r(OTs[0][0:64, ts(t2, 128)]), r(wps[0][:]),
                    start=True, stop=True,
                )
                pr1 = pspr.tile([128, 512], F32, tag="pr")
                nc.tensor.matmul(
                    pr1[:], r(OTs[1][0:64, ts(t2, 128)]), r(wps[1][:]),
                    start=True, stop=True,
                )
                tmp = comb.tile([128, 512], F32, tag="tmp")
                nc.scalar.mul(tmp[:], pr0[:], rl[:, t2 : t2 + 1])
                ob = comb.tile([128, 512], F32, tag="ob")
                nc.vector.scalar_tensor_tensor(
                    ob[:], pr1[:], rl[:, 16 + t2 : 16 + t2 + 1], tmp[:], mult, add
                )
                nc.sync.dma_start(out=out_d[ts(t2, 128), :], in_=ob[:])
        if PHASE_LIMIT == "D":
            _dump(nc, out_d, OTs[0], 0)
            _dump(nc, out_d, OTs[1], 1024)


_PROGRAM_CACHE = {}
TRACE = False
_LAST = {}


def _build_program():
    key = ("prog", USE_F32R, PHASE_LIMIT, GRP, DBG_HEADS, DBG_NOMASK)
    if key in _PROGRAM_CACHE:
        return _PROGRAM_CACHE[key]
    from contextlib import ExitStack

    WDT = F32R if USE_F32R else F32

    nc = bacc.Bacc(trn_type="TRN2", target_bir_lowering=False, debug=False,
                   num_devices=NCORES)
    in_d = {
        "x": nc.dram_tensor("x", [T, C], WDT, kind="ExternalInput").ap(),
        "wq": nc.dram_tensor("wq", [C, 128], WDT, kind="ExternalInput").ap(),
        "wk": nc.dram_tensor("wk", [C, 128], WDT, kind="ExternalInput").ap(),
        "wv": nc.dram_tensor("wv", [C, 128], WDT, kind="ExternalInput").ap(),
        "bq": nc.dram_tensor("bq", [1, 128], WDT, kind="ExternalInput").ap(),
        "bk": nc.dram_tensor("bk", [1, 128], WDT, kind="ExternalInput").ap(),
        "bv": nc.dram_tensor("bv", [1, 128], WDT, kind="ExternalInput").ap(),
        "wp": nc.dram_tensor("wp", [128, C], WDT, kind="ExternalInput").ap(),
        "ident": nc.dram_tensor("ident", [128, 128], WDT, kind="ExternalInput").ap(),
        "identh": nc.dram_tensor("identh", [128, 64], WDT, kind="ExternalInput").ap(),
        "mask_k": nc.dram_tensor("mask_k", [64, T], WDT, kind="ExternalInput").ap(),
        "mask_q": nc.dram_tensor("mask_q", [64, T], WDT, kind="ExternalInput").ap(),
        "sel65": nc.dram_tensor("sel65", [65, 1], F32, kind="ExternalInput").ap(),
        "ones_row": nc.dram_tensor("ones_row", [1, T], WDT, kind="ExternalInput").ap(),
        "ones_col": nc.dram_tensor("ones_col", [128, 1], WDT, kind="ExternalInput").ap(),
    }
    out_d = nc.dram_tensor("out", [T, C], F32, kind="ExternalOutput").ap()
    with tile.TileContext(nc) as tc:
        with ExitStack() as ctx:
            _emit(ctx, tc, out_d, in_d)
    nc.compile()
    _PROGRAM_CACHE[key] = nc
    return nc


def _consts():
    f = np.float32
    ident = np.eye(128, dtype=f)
    rr = np.arange(64)
    jj = np.arange(T)
    # mask_k[r, j] = 1 where r == 32*(jb%2) + (j%128)//4 (jb = j//128)
    mk = (rr[:, None] == 32 * ((jj[None, :] // 128) % 2) + (jj[None, :] % 128) // NA)
    mask_k = mk.astype(f)
    # mask_q[r, i] = NEG where (i%256)//4 < r
    mask_q = np.where((jj[None, :] % IC) // NA < rr[:, None], f(NEG), f(0.0)).astype(f)
    sel = np.zeros((65, 1), dtype=f)
    sel[64, 0] = 1.0
    identh = np.vstack([np.eye(64, dtype=f), np.eye(64, dtype=f)])
    return ident, mask_k, mask_q, sel, identh


def _numpy_reference(x, mask, Wq, bq, Wk, bk, Wv, bv, Wp, bp):
    b, t, c = x.shape
    h, d = H, c // H
    scale = 1.0 / math.sqrt(d)
    tril = np.tril(np.ones((NF, NF), dtype=np.float32))
    block = np.kron(tril, np.ones((NA, NA), dtype=np.float32))

    def heads(w, bias):
        return (x @ w + bias).reshape(b, t, h, d).transpose(0, 2, 1, 3)

    q, k, v = heads(Wq, bq), heads(Wk, bk), heads(Wv, bv)
    att = np.einsum("bhqd,bhkd->bhqk", q, k) * scale
    allowed = block[None, None] * mask[:, None, None, :].astype(np.float32)
    att = np.where(allowed == 0, -np.inf, att)
    att = att - att.max(axis=-1, keepdims=True)
    att = np.exp(att)
    att = att / att.sum(axis=-1, keepdims=True)
    y = np.einsum("bhqk,bhkd->bhqd", att, v)
    y = y.transpose(0, 2, 1, 3).reshape(b, t, c)
    return (y @ Wp + bp).astype(np.float32)


def kernel(**inputs):
    x = np.asarray(inputs["x"], dtype=np.float32)
    mask = np.asarray(inputs["mask"])
    Wq = np.asarray(inputs["Wq"], dtype=np.float32)
    bq = np.asarray(inputs["bq"], dtype=np.float32)
    Wk = np.asarray(inputs["Wk"], dtype=np.float32)
    bk = np.asarray(inputs["bk"], dtype=np.float32)
    Wv = np.asarray(inputs["Wv"], dtype=np.float32)
    bv = np.asarray(inputs["bv"], dtype=np.float32)
    Wp = np.asarray(inputs["Wp"], dtype=np.float32)
    bp = np.asarray(inputs["bp"], dtype=np.float32)

    if not np.all(np.asarray(mask) == 1):
        return _numpy_reference(x, mask, Wq, bq, Wk, bk, Wv, bv, Wp, bp)

    nc = _build_program()
    ident, mask_k, mask_q, sel, identh = _consts()
    in_maps = []
    for core in range(NCORES):
        b = core // CPB
        hp = core % CPB
        cs = slice(hp * 128, (hp + 1) * 128)
        in_maps.append({
            "x": np.ascontiguousarray(x[b]),
            "wq": np.ascontiguousarray(Wq[:, cs]) * np.float32(SCALE),
            "wk": np.ascontiguousarray(Wk[:, cs]),
            "wv": np.ascontiguousarray(Wv[:, cs]),
            "bq": (bq[cs] * np.float32(SCALE)).reshape(1, 128).copy(),
            "bk": bk[cs].reshape(1, 128).copy(),
            "bv": bv[cs].reshape(1, 128).copy(),
            "wp": np.ascontiguousarray(Wp[cs, :]),
            "ident": ident,
            "identh": identh,
            "mask_k": mask_k,
            "mask_q": mask_q,
            "sel65": sel,
            "ones_row": np.ones((1, T), dtype=np.float32),
            "ones_col": np.ones((128, 1), dtype=np.float32),
        })
    rr = run_bass_kernel_spmd(
        nc, in_maps, list(range(NCORES)), trace=TRACE,
        tmpdir=_LAST.get("tmpdir"),
    )
    _LAST["results"] = rr
    res = rr.results
    out = np.zeros((B, T, C), dtype=np.float32)
    for b in range(B):
        acc = res[b * CPB]["out"].astype(np.float32)
        for j in range(1, CPB):
            acc = acc + res[b * CPB + j]["out"]
        out[b] = acc + bp[None, :]
    return out



# revision 3
# speedup vs baseline: 1.3947x; 1.3947x over previous
"""Block-causal multi-head self-attention for TRN2, sharded over 8 NeuronCores.

Problem (hardcoded): B=2, T=2048 (512 frames x 4 animals), C=512, H=8 heads,
D=64. Block-causal mask = kron(tril(frames), ones(4,4)); key mask is all-ones
per the input spec (a numpy fallback handles the general case).

Sharding: core i handles batch b = i//4 and heads (2*(i%4), 2*(i%4)+1).
Wq/Wk/Wv are column-sharded (128 cols per core), Wp row-sharded (128 rows per
core). Each core emits a partial [T, C] output projection; the host sums the
4 partials per batch and adds bp.

On-core algorithm (bf16 matmul data, fp32 PSUM accumulation):
  xT arrives pre-transposed from the host ([C, T] bf16)
  qT/kT/vT = W.T @ xT (+bias via K=1 matmul; softmax scale folded into Wq)
  per head h, per 256-wide query chunk qc:
    S^T[j,i] = kT^T(128-j-block) @ qT(chunk)    (K=64, blocks j<=diag)
    diagonal tri-mask added via a rank-64 indicator matmul (-1e9 offside)
    P^T = exp(S^T)  (ACT, PSUM->SBUF bf16, groups of 4 j-blocks)
    O^T_aug[65,256] += V_aug(block).T @ P^T     (V col 64 = ones -> row 64 = l)
  l column per 128-row t-tile via a [65,1] selector matmul; rl = 1/l
  partial = (O^T_h[:64].T @ Wp_h) * rl_h summed over the 2 heads (ACT/DVE)
"""

import math

import numpy as np
import ml_dtypes

import concourse.bass as bass
import concourse.bacc as bacc
import concourse.tile as tile
from concourse import mybir
from concourse.bass_utils import run_bass_kernel_spmd

B, T, C, H, D = 2, 2048, 512, 8, 64
NF, NA = 512, 4
NCORES = 8
HPC = 2            # heads per core
CPB = 4            # cores per batch
SCALE = 1.0 / math.sqrt(D)
NEG = -1.0e9
IC = 256           # query-chunk width
NQC = T // IC      # 8
GRP = 4            # j-blocks (128 keys each) per exp group

F32 = mybir.dt.float32
BF16 = mybir.dt.bfloat16
NPBF16 = ml_dtypes.bfloat16

PHASE_LIMIT = None   # None | "B" | "C" | "D" (debug: stop after phase)
DBG_HEADS = None     # debug: restrict head loop
DBG_NOMASK = False   # debug: skip diagonal mask matmuls


def _dump(nc, out_d, t, row0, ncols=512):
    p, fsz = t.shape[0], t.shape[1]
    n = min(fsz // ncols, (T - row0) // 128)
    for i in range(n):
        nc.sync.dma_start(
            out=out_d[row0 + i * 128 : row0 + i * 128 + p, 0:ncols],
            in_=t[:, i * ncols : (i + 1) * ncols].bitcast(F32),
        )


def _emit(ctx, tc, out_d, in_d):
    nc = tc.nc
    WDT = BF16
    ts = bass.ts
    Exp = mybir.ActivationFunctionType.Exp
    mult, add = mybir.AluOpType.mult, mybir.AluOpType.add

    const = ctx.enter_context(tc.tile_pool(name="const", bufs=1))
    big = ctx.enter_context(tc.tile_pool(name="big", bufs=1))

    sel = const.tile([65, 1], WDT)
    nc.sync.dma_start(out=sel[:], in_=in_d["sel65"][:])
    identh = const.tile([128, 64], WDT)
    nc.sync.dma_start(out=identh[:], in_=in_d["identh"][:])

    w_sb, b_sb = {}, {}
    for nm in ("wq", "wk", "wv"):
        w = const.tile([128, 512], WDT, tag=f"w_{nm}")
        for cb in range(4):
            nc.sync.dma_start(out=w[:, ts(cb, 128)], in_=in_d[nm][ts(cb, 128), :])
        w_sb[nm] = w
        bt = const.tile([1, 128], WDT, tag=f"b_{nm}")
        nc.sync.dma_start(out=bt[:], in_=in_d["b" + nm[1]][:])
        b_sb[nm] = bt
    wps = []
    for h in range(HPC):
        wph = const.tile([64, 512], WDT, tag=f"wp{h}", name=f"wp{h}")
        nc.sync.dma_start(out=wph[:], in_=in_d["wp"][ts(h, 64), :])
        wps.append(wph)

    ones_row = const.tile([1, T], WDT, tag="ones")
    nc.sync.dma_start(out=ones_row[:], in_=in_d["ones_row"][:])
    ones_col = const.tile([128, 1], WDT, tag="ones_col")
    nc.sync.dma_start(out=ones_col[:], in_=in_d["ones_col"][:])

    xT = big.tile([128, 4 * T], WDT, tag="xT")
    # xT arrives pre-transposed [C, T] from the host: 4 partition chunks.
    for cb in range(4):
        nc.sync.dma_start(
            out=xT[:, cb * T : (cb + 1) * T], in_=in_d["xt"][ts(cb, 128), :]
        )
    qA = [big.tile([128, T], WDT, tag=f"qA{h}", name=f"qA{h}") for h in range(HPC)]
    kA = [big.tile([128, T], WDT, tag=f"kA{h}", name=f"kA{h}") for h in range(HPC)]
    vT = big.tile([128, T], WDT, tag="vT")
    # mask rows: head0 tiles carry them in partitions 64:128, head1 in 0:64
    nc.sync.dma_start(out=kA[0][64:128, :], in_=in_d["mask_k"][:])
    nc.sync.dma_start(out=kA[1][0:64, :], in_=in_d["mask_k"][:])
    nc.sync.dma_start(out=qA[0][64:128, :], in_=in_d["mask_q"][:])
    nc.sync.dma_start(out=qA[1][0:64, :], in_=in_d["mask_q"][:])
    Vb = big.tile([128, HPC * 16 * 65], WDT, tag="Vb")
    OTs = [big.tile([65, T], WDT, tag=f"OT{h}", name=f"OT{h}") for h in range(HPC)]
    rl = big.tile([128, HPC * 16], F32, tag="rl")

    # Phase B: projections -> qT/kT/vT in [head_ch, t] layout.
    with tc.tile_pool(name="psproj", bufs=2, space="PSUM") as psp:
        for nm, dst in (("wq", qA), ("wk", kA), ("wv", vT)):
            for t5 in range(4):
                ps = psp.tile([128, 512], F32)
                for cb in range(4):
                    nc.tensor.matmul(
                        ps[:],
                        w_sb[nm][:, ts(cb, 128)],
                        xT[:, cb * T + t5 * 512 : cb * T + (t5 + 1) * 512],
                        start=(cb == 0),
                        stop=False,
                    )
                nc.tensor.matmul(
                    ps[:], b_sb[nm][:], ones_row[:, ts(t5, 512)],
                    start=False, stop=True,
                )
                if nm == "wv":
                    nc.vector.tensor_copy(dst[:, ts(t5, 512)], ps[:])
                else:
                    nc.vector.tensor_copy(dst[0][0:64, ts(t5, 512)], ps[0:64, :])
                    nc.vector.tensor_copy(dst[1][64:128, ts(t5, 512)], ps[64:128, :])

    if PHASE_LIMIT == "B":
        _dump(nc, out_d, qA[0], 0)
        _dump(nc, out_d, kA[0], 512)
        _dump(nc, out_d, vT, 1024)
        return

    # Phase C: V_aug blocks [128 j, 65] per (head, j-block); col 64 = ones.
    with tc.tile_pool(name="psv", bufs=4, space="PSUM") as psv:
        for h in range(HPC):
            for jb in range(16):
                pv = psv.tile([128, 64], WDT)
                nc.tensor.transpose(
                    pv[:], vT[ts(h, 64), ts(jb, 128)], identh[ts(h, 64), :]
                )
                base = (h * 16 + jb) * 65
                nc.vector.tensor_copy(Vb[:, base : base + 64], pv[:])
                nc.vector.tensor_copy(Vb[:, base + 64 : base + 65], ones_col[:])

    if PHASE_LIMIT == "C":
        _dump(nc, out_d, Vb, 0)
        return

    # Phase D/E: attention + projection, interleaved per query chunk.
    with tc.tile_pool(name="pss", bufs=2, space="PSUM") as pss, \
         tc.tile_pool(name="psot", bufs=1, space="PSUM") as psot, \
         tc.tile_pool(name="psl", bufs=1, space="PSUM") as pslp, \
         tc.tile_pool(name="pspr", bufs=2, space="PSUM") as pspr, \
         tc.tile_pool(name="ptp", bufs=3) as ptp, \
         tc.tile_pool(name="comb", bufs=4) as comb:
        psl = pslp.tile([128, HPC * 16], F32)
        for qc in range(NQC):
            nbj = 2 * (qc + 1)
            for h in range(DBG_HEADS or HPC):
                hs = slice(0, 64) if h == 0 else slice(64, 128)
                ot = psot.tile([65, IC], F32)
                for g0 in range(0, nbj, GRP):
                    ng = min(GRP, nbj - g0)
                    ps = pss.tile([128, GRP * IC], F32)
                    for k in range(ng):
                        jb = g0 + k
                        sl = ps[:, ts(k, IC)]
                        diag = (jb >= 2 * qc) and not DBG_NOMASK
                        if diag:
                            nc.tensor.matmul(
                                sl,
                                kA[h][:, ts(jb, 128)],
                                qA[h][:, ts(qc, IC)],
                                start=True, stop=True,
                            )
                        else:
                            nc.tensor.matmul(
                                sl,
                                kA[h][hs, ts(jb, 128)],
                                qA[h][hs, ts(qc, IC)],
                                start=True, stop=True,
                            )
                    ptt = ptp.tile([128, GRP * IC], WDT)
                    nc.scalar.activation(ptt[:, 0 : ng * IC], ps[:, 0 : ng * IC], Exp)
                    for k in range(ng):
                        jb = g0 + k
                        vbase = (h * 16 + jb) * 65
                        nc.tensor.matmul(
                            ot[:],
                            Vb[:, vbase : vbase + 65],
                            ptt[:, ts(k, IC)],
                            start=(jb == 0),
                            stop=(jb == nbj - 1),
                        )
                nc.vector.tensor_copy(OTs[h][:, ts(qc, IC)], ot[:])
            if PHASE_LIMIT == "D":
                continue
            for t2 in (2 * qc, 2 * qc + 1):
                for h in range(HPC):
                    col = h * 16 + t2
                    nc.tensor.matmul(
                        psl[:, col : col + 1],
                        OTs[h][:, ts(t2, 128)],
                        sel[:],
                        start=True,
                        stop=True,
                    )
                    nc.vector.reciprocal(rl[:, col : col + 1], psl[:, col : col + 1])
                pr0 = pspr.tile([128, 512], F32, tag="pr")
                nc.tensor.matmul(
                    pr0[:], OTs[0][0:64, ts(t2, 128)], wps[0][:],
                    start=True, stop=True,
                )
                pr1 = pspr.tile([128, 512], F32, tag="pr")
                nc.tensor.matmul(
                    pr1[:], OTs[1][0:64, ts(t2, 128)], wps[1][:],
                    start=True, stop=True,
                )
                tmp = comb.tile([128, 512], F32, tag="tmp")
                nc.scalar.mul(tmp[:], pr0[:], rl[:, t2 : t2 + 1])
                ob = comb.tile([128, 512], F32, tag="ob")
                nc.vector.scalar_tensor_tensor(
                    ob[:], pr1[:], rl[:, 16 + t2 : 16 + t2 + 1], tmp[:], mult, add
                )
                nc.sync.dma_start(out=out_d[ts(t2, 128), :], in_=ob[:])
        if PHASE_LIMIT == "D":
            _dump(nc, out_d, OTs[0], 0)
            _dump(nc, out_d, OTs[1], 1024)


_PROGRAM_CACHE = {}
TRACE = False
_LAST = {}


def _build_program():
    key = ("prog", PHASE_LIMIT, GRP, DBG_HEADS, DBG_NOMASK)
    if key in _PROGRAM_CACHE:
        return _PROGRAM_CACHE[key]
    from contextlib import ExitStack

    WDT = BF16

    nc = bacc.Bacc(trn_type="TRN2", target_bir_lowering=False, debug=False,
                   num_devices=NCORES)
    in_d = {
        "xt": nc.dram_tensor("xt", [C, T], WDT, kind="ExternalInput").ap(),
        "wq": nc.dram_tensor("wq", [C, 128], WDT, kind="ExternalInput").ap(),
        "wk": nc.dram_tensor("wk", [C, 128], WDT, kind="ExternalInput").ap(),
        "wv": nc.dram_tensor("wv", [C, 128], WDT, kind="ExternalInput").ap(),
        "bq": nc.dram_tensor("bq", [1, 128], WDT, kind="ExternalInput").ap(),
        "bk": nc.dram_tensor("bk", [1, 128], WDT, kind="ExternalInput").ap(),
        "bv": nc.dram_tensor("bv", [1, 128], WDT, kind="ExternalInput").ap(),
        "wp": nc.dram_tensor("wp", [128, C], WDT, kind="ExternalInput").ap(),
        "identh": nc.dram_tensor("identh", [128, 64], WDT, kind="ExternalInput").ap(),
        "mask_k": nc.dram_tensor("mask_k", [64, T], WDT, kind="ExternalInput").ap(),
        "mask_q": nc.dram_tensor("mask_q", [64, T], WDT, kind="ExternalInput").ap(),
        "sel65": nc.dram_tensor("sel65", [65, 1], WDT, kind="ExternalInput").ap(),
        "ones_row": nc.dram_tensor("ones_row", [1, T], WDT, kind="ExternalInput").ap(),
        "ones_col": nc.dram_tensor("ones_col", [128, 1], WDT, kind="ExternalInput").ap(),
    }
    out_d = nc.dram_tensor("out", [T, C], F32, kind="ExternalOutput").ap()
    with tile.TileContext(nc) as tc:
        with ExitStack() as ctx:
            _emit(ctx, tc, out_d, in_d)
    nc.compile()
    _PROGRAM_CACHE[key] = nc
    return nc


def _consts():
    f = np.float32
    rr = np.arange(64)
    jj = np.arange(T)
    # mask_k[r, j] = 1 where r == 32*(jb%2) + (j%128)//4 (jb = j//128)
    mk = (rr[:, None] == 32 * ((jj[None, :] // 128) % 2) + (jj[None, :] % 128) // NA)
    mask_k = mk.astype(f)
    # mask_q[r, i] = NEG where (i%256)//4 < r
    mask_q = np.where((jj[None, :] % IC) // NA < rr[:, None], f(NEG), f(0.0)).astype(f)
    sel = np.zeros((65, 1), dtype=f)
    sel[64, 0] = 1.0
    identh = np.vstack([np.eye(64, dtype=f), np.eye(64, dtype=f)])
    return mask_k, mask_q, sel, identh


def _numpy_reference(x, mask, Wq, bq, Wk, bk, Wv, bv, Wp, bp):
    b, t, c = x.shape
    h, d = H, c // H
    scale = 1.0 / math.sqrt(d)
    tril = np.tril(np.ones((NF, NF), dtype=np.float32))
    block = np.kron(tril, np.ones((NA, NA), dtype=np.float32))

    def heads(w, bias):
        return (x @ w + bias).reshape(b, t, h, d).transpose(0, 2, 1, 3)

    q, k, v = heads(Wq, bq), heads(Wk, bk), heads(Wv, bv)
    att = np.einsum("bhqd,bhkd->bhqk", q, k) * scale
    allowed = block[None, None] * mask[:, None, None, :].astype(np.float32)
    att = np.where(allowed == 0, -np.inf, att)
    att = att - att.max(axis=-1, keepdims=True)
    att = np.exp(att)
    att = att / att.sum(axis=-1, keepdims=True)
    y = np.einsum("bhqk,bhkd->bhqd", att, v)
    y = y.transpose(0, 2, 1, 3).reshape(b, t, c)
    return (y @ Wp + bp).astype(np.float32)


def kernel(**inputs):
    x = np.asarray(inputs["x"], dtype=np.float32)
    mask = np.asarray(inputs["mask"])
    Wq = np.asarray(inputs["Wq"], dtype=np.float32)
    bq = np.asarray(inputs["bq"], dtype=np.float32)
    Wk = np.asarray(inputs["Wk"], dtype=np.float32)
    bk = np.asarray(inputs["bk"], dtype=np.float32)
    Wv = np.asarray(inputs["Wv"], dtype=np.float32)
    bv = np.asarray(inputs["bv"], dtype=np.float32)
    Wp = np.asarray(inputs["Wp"], dtype=np.float32)
    bp = np.asarray(inputs["bp"], dtype=np.float32)

    if not np.all(np.asarray(mask) == 1):
        return _numpy_reference(x, mask, Wq, bq, Wk, bk, Wv, bv, Wp, bp)

    nc = _build_program()
    mask_k, mask_q, sel, identh = _consts()
    bf = NPBF16
    in_maps = []
    for core in range(NCORES):
        b = core // CPB
        hp = core % CPB
        cs = slice(hp * 128, (hp + 1) * 128)
        in_maps.append({
            "xt": np.ascontiguousarray(x[b].T).astype(bf),
            "wq": (np.ascontiguousarray(Wq[:, cs]) * np.float32(SCALE)).astype(bf),
            "wk": np.ascontiguousarray(Wk[:, cs]).astype(bf),
            "wv": np.ascontiguousarray(Wv[:, cs]).astype(bf),
            "bq": (bq[cs] * np.float32(SCALE)).reshape(1, 128).astype(bf),
            "bk": bk[cs].reshape(1, 128).astype(bf),
            "bv": bv[cs].reshape(1, 128).astype(bf),
            "wp": np.ascontiguousarray(Wp[cs, :]).astype(bf),
            "identh": identh.astype(bf),
            "mask_k": mask_k.astype(bf),
            "mask_q": mask_q.astype(bf),
            "sel65": sel.astype(bf),
            "ones_row": np.ones((1, T), dtype=bf),
            "ones_col": np.ones((128, 1), dtype=bf),
        })
    rr = run_bass_kernel_spmd(
        nc, in_maps, list(range(NCORES)), trace=TRACE,
        tmpdir=_LAST.get("tmpdir"),
    )
    _LAST["results"] = rr
    res = rr.results
    out = np.zeros((B, T, C), dtype=np.float32)
    for b in range(B):
        acc = res[b * CPB]["out"].astype(np.float32)
        for j in range(1, CPB):
            acc = acc + res[b * CPB + j]["out"]
        out[b] = acc + bp[None, :]
    return out


# revision 10
# speedup vs baseline: 1.4555x; 1.0436x over previous
"""Block-causal multi-head self-attention for TRN2, sharded over 8 NeuronCores.

Problem (hardcoded): B=2, T=2048 (512 frames x 4 animals), C=512, H=8 heads,
D=64. Block-causal mask = kron(tril(frames), ones(4,4)); key mask is all-ones
per the input spec (a numpy fallback handles the general case).

Sharding: core i handles batch b = i//4 and heads (2*(i%4), 2*(i%4)+1).
Wq/Wk/Wv are column-sharded (128 cols per core), Wp row-sharded (128 rows per
core). Each core emits a partial [T, C] output projection; the host sums the
4 partials per batch and adds bp.

On-core algorithm (bf16 matmul data, fp32 PSUM accumulation):
  xT arrives pre-transposed from the host ([C, T] bf16)
  qT/kT/vT = W.T @ xT (+bias via K=1 matmul; softmax scale folded into Wq)
  per head h, per 256-wide query chunk qc:
    S^T[j,i] = kT^T(128-j-block) @ qT(chunk)    (K=64, blocks j<=diag)
    diagonal tri-mask added via a rank-64 indicator matmul (-1e9 offside)
    P^T = exp(S^T)  (ACT, PSUM->SBUF bf16, groups of 4 j-blocks)
    O^T_aug[65,256] += V_aug(block).T @ P^T     (V col 64 = ones -> row 64 = l)
  l column per 128-row t-tile via a [65,1] selector matmul; rl = 1/l
  partial = (O^T_h[:64].T @ Wp_h) * rl_h summed over the 2 heads (ACT/DVE)
"""

import math

import numpy as np
import ml_dtypes

import concourse.bass as bass
import concourse.bacc as bacc
import concourse.tile as tile
from concourse import mybir
from concourse.bass_utils import run_bass_kernel_spmd

B, T, C, H, D = 2, 2048, 512, 8, 64
NF, NA = 512, 4
NCORES = 8
HPC = 2            # heads per core
CPB = 4            # cores per batch
SCALE = 1.0 / math.sqrt(D)
NEG = -1.0e9
IC = 256           # query-chunk width
NQC = T // IC      # 8
GRP = 4            # j-blocks (128 keys each) per exp group

F32 = mybir.dt.float32
BF16 = mybir.dt.bfloat16
NPBF16 = ml_dtypes.bfloat16

PHASE_LIMIT = None   # None | "B" | "C" | "D" (debug: stop after phase)
DBG_HEADS = None     # debug: restrict head loop
DBG_NOMASK = False   # debug: skip diagonal mask matmuls


def _dump(nc, out_d, t, row0, ncols=512):
    p, fsz = t.shape[0], t.shape[1]
    n = min(fsz // ncols, (T - row0) // 128)
    for i in range(n):
        nc.sync.dma_start(
            out=out_d[row0 + i * 128 : row0 + i * 128 + p, 0:ncols],
            in_=t[:, i * ncols : (i + 1) * ncols].bitcast(F32),
        )


def _emit(ctx, tc, out_d, in_d):
    nc = tc.nc
    WDT = BF16
    ts = bass.ts
    Exp = mybir.ActivationFunctionType.Exp
    mult, add = mybir.AluOpType.mult, mybir.AluOpType.add

    const = ctx.enter_context(tc.tile_pool(name="const", bufs=1))
    big = ctx.enter_context(tc.tile_pool(name="big", bufs=1))

    sel = const.tile([65, 1], WDT)
    nc.sync.dma_start(out=sel[:], in_=in_d["sel65"][:])
    identh = const.tile([128, 64], WDT)
    nc.sync.dma_start(out=identh[:], in_=in_d["identh"][:])

    w_sb, b_sb = {}, {}
    for nm in ("wq", "wk", "wv"):
        w = const.tile([128, 512], WDT, tag=f"w_{nm}")
        for cb in range(4):
            nc.sync.dma_start(out=w[:, ts(cb, 128)], in_=in_d[nm][ts(cb, 128), :])
        w_sb[nm] = w
        bt = const.tile([1, 128], WDT, tag=f"b_{nm}")
        nc.sync.dma_start(out=bt[:], in_=in_d["b" + nm[1]][:])
        b_sb[nm] = bt
    wps = []
    for h in range(HPC):
        wph = const.tile([64, 512], WDT, tag=f"wp{h}", name=f"wp{h}")
        nc.sync.dma_start(out=wph[:], in_=in_d["wp"][ts(h, 64), :])
        wps.append(wph)

    ones_row = const.tile([1, T], WDT, tag="ones")
    nc.sync.dma_start(out=ones_row[:], in_=in_d["ones_row"][:])
    ones_col = const.tile([128, 1], WDT, tag="ones_col")
    nc.sync.dma_start(out=ones_col[:], in_=in_d["ones_col"][:])

    xT = big.tile([128, 4 * T], WDT, tag="xT")
    # xT arrives pre-transposed [C, T] from the host. 16 chunks, earliest-
    # needed first, so Phase B starts before the full 2MB lands.
    for t5 in range(4):
        for cb in range(4):
            nc.sync.dma_start(
                out=xT[:, cb * T + t5 * 512 : cb * T + (t5 + 1) * 512],
                in_=in_d["xt"][ts(cb, 128), ts(t5, 512)],
            )
    qA = [big.tile([128, T], WDT, tag=f"qA{h}", name=f"qA{h}") for h in range(HPC)]
    kA = [big.tile([128, T], WDT, tag=f"kA{h}", name=f"kA{h}") for h in range(HPC)]
    vT = big.tile([128, T], WDT, tag="vT")
    # mask rows: head0 tiles carry them in partitions 64:128, head1 in 0:64
    nc.sync.dma_start(out=kA[0][64:128, :], in_=in_d["mask_k"][:])
    nc.sync.dma_start(out=kA[1][0:64, :], in_=in_d["mask_k"][:])
    nc.sync.dma_start(out=qA[0][64:128, :], in_=in_d["mask_q"][:])
    nc.sync.dma_start(out=qA[1][0:64, :], in_=in_d["mask_q"][:])
    Vb = big.tile([128, HPC * 16 * 65], WDT, tag="Vb")
    OTs = [big.tile([65, T], WDT, tag=f"OT{h}", name=f"OT{h}") for h in range(HPC)]
    rl = big.tile([128, HPC * 16], F32, tag="rl")

    # Phase B: projections -> qT/kT/vT in [head_ch, t] layout.
    with tc.tile_pool(name="psproj", bufs=2, space="PSUM") as psp:
        for nm, dst in (("wq", qA), ("wk", kA), ("wv", vT)):
            for t5 in range(4):
                ps = psp.tile([128, 512], F32)
                for cb in range(4):
                    nc.tensor.matmul(
                        ps[:],
                        w_sb[nm][:, ts(cb, 128)],
                        xT[:, cb * T + t5 * 512 : cb * T + (t5 + 1) * 512],
                        start=(cb == 0),
                        stop=False,
                    )
                nc.tensor.matmul(
                    ps[:], b_sb[nm][:], ones_row[:, ts(t5, 512)],
                    start=False, stop=True,
                )
                if nm == "wv":
                    nc.vector.tensor_copy(dst[:, ts(t5, 512)], ps[:])
                else:
                    nc.vector.tensor_copy(dst[0][0:64, ts(t5, 512)], ps[0:64, :])
                    nc.vector.tensor_copy(dst[1][64:128, ts(t5, 512)], ps[64:128, :])

    if PHASE_LIMIT == "B":
        _dump(nc, out_d, qA[0], 0)
        _dump(nc, out_d, kA[0], 512)
        _dump(nc, out_d, vT, 1024)
        return

    # Phase C: V_aug blocks [128 j, 65] per (head, j-block); col 64 = ones.
    with tc.tile_pool(name="psv", bufs=4, space="PSUM") as psv:
        for h in range(HPC):
            for jb in range(16):
                pv = psv.tile([128, 64], WDT)
                nc.tensor.transpose(
                    pv[:], vT[ts(h, 64), ts(jb, 128)], identh[ts(h, 64), :]
                )
                base = (h * 16 + jb) * 65
                nc.scalar.copy(Vb[:, base : base + 64], pv[:])
                nc.gpsimd.tensor_copy(Vb[:, base + 64 : base + 65], ones_col[:])

    if PHASE_LIMIT == "C":
        _dump(nc, out_d, Vb, 0)
        return

    # Phase D/E: attention + projection, software-pipelined across (qc, h,
    # group) so the PE always has the next S group queued while the scalar
    # engine runs exp on the previous one.
    bypass = mybir.AluOpType.bypass
    with tc.tile_pool(name="pss", bufs=2, space="PSUM") as pss, \
         tc.tile_pool(name="psot", bufs=1, space="PSUM") as psot, \
         tc.tile_pool(name="prl", bufs=3, space="PSUM") as prl, \
         tc.tile_pool(name="ptp", bufs=3) as ptp, \
         tc.tile_pool(name="comb", bufs=4) as comb:
        flat = []
        for qc in range(NQC):
            nbj = 2 * (qc + 1)
            for h in range(DBG_HEADS or HPC):
                for g0 in range(0, nbj, GRP):
                    flat.append((qc, h, g0, min(GRP, nbj - g0), nbj))
        n = len(flat)
        ps_t = {}
        ot_t = {}

        def emit_s(i):
            qc, h, g0, ng, nbj = flat[i]
            ps = pss.tile([128, GRP * IC], F32, tag="ps", name=f"ps{i}")
            ps_t[i] = ps
            hs = slice(0, 64) if h == 0 else slice(64, 128)
            for k in range(ng):
                jb = g0 + k
                sl = ps[:, ts(k, IC)]
                if jb >= 2 * qc and not DBG_NOMASK:
                    nc.tensor.matmul(
                        sl, kA[h][:, ts(jb, 128)], qA[h][:, ts(qc, IC)],
                        start=True, stop=True,
                    )
                else:
                    nc.tensor.matmul(
                        sl, kA[h][hs, ts(jb, 128)], qA[h][hs, ts(qc, IC)],
                        start=True, stop=True,
                    )

        def emit_tail(qc):
            for t2 in (2 * qc, 2 * qc + 1):
                pp = prl.tile([128, 512], F32, tag="pr", name=f"pp{t2}")
                for h in range(HPC):
                    nc.tensor.matmul(
                        pp[:, h : h + 1], OTs[h][:, ts(t2, 128)], sel[:],
                        start=True, stop=True,
                    )
                nc.vector.reciprocal(rl[:, 2 * t2 : 2 * t2 + 2], pp[:, 0:2])
                pr0 = prl.tile([128, 512], F32, tag="pr", name=f"pr0_{t2}")
                nc.tensor.matmul(
                    pr0[:], OTs[0][0:64, ts(t2, 128)], wps[0][:],
                    start=True, stop=True,
                )
                pr1 = prl.tile([128, 512], F32, tag="pr", name=f"pr1_{t2}")
                nc.tensor.matmul(
                    pr1[:], OTs[1][0:64, ts(t2, 128)], wps[1][:],
                    start=True, stop=True,
                )
                tmp = comb.tile([128, 512], F32, tag="tmp")
                nc.vector.tensor_scalar_mul(
                    tmp[:], pr0[:], rl[:, 2 * t2 : 2 * t2 + 1]
                )
                ob = comb.tile([128, 512], WDT, tag="ob")
                nc.vector.scalar_tensor_tensor(
                    ob[:], pr1[:], rl[:, 2 * t2 + 1 : 2 * t2 + 2], tmp[:], mult, add
                )
                nc.sync.dma_start(out=out_d[ts(t2, 128), :], in_=ob[:])

        emit_s(0)
        if n > 1:
            emit_s(1)
        for i in range(n):
            qc, h, g0, ng, nbj = flat[i]
            ps = ps_t.pop(i)
            ptt = ptp.tile([128, GRP * IC], WDT)
            nc.scalar.activation(ptt[:, 0 : ng * IC], ps[:, 0 : ng * IC], Exp)
            if g0 == 0 and h == 0:
                ot_t[qc] = psot.tile([65, HPC * IC], F32, tag="ot", name=f"ot{qc}")
            ot = ot_t[qc]
            osl = ot[:, h * IC : (h + 1) * IC]
            for k in range(ng):
                jb = g0 + k
                vbase = (h * 16 + jb) * 65
                nc.tensor.matmul(
                    osl, Vb[:, vbase : vbase + 65], ptt[:, ts(k, IC)],
                    start=(jb == 0), stop=(jb == nbj - 1),
                )
            if i + 2 < n:
                emit_s(i + 2)
            if g0 + ng == nbj:
                nc.vector.tensor_copy(OTs[h][:, ts(qc, IC)], osl)
                if h == HPC - 1 and PHASE_LIMIT != "D":
                    emit_tail(qc)
        if PHASE_LIMIT == "D":
            _dump(nc, out_d, OTs[0], 0)
            _dump(nc, out_d, OTs[1], 1024)


_PROGRAM_CACHE = {}
TRACE = False
_LAST = {}


def _build_program():
    key = ("prog", PHASE_LIMIT, GRP, DBG_HEADS, DBG_NOMASK)
    if key in _PROGRAM_CACHE:
        return _PROGRAM_CACHE[key]
    from contextlib import ExitStack

    WDT = BF16

    nc = bacc.Bacc(trn_type="TRN2", target_bir_lowering=False, debug=False,
                   num_devices=NCORES)
    in_d = {
        "xt": nc.dram_tensor("xt", [C, T], WDT, kind="ExternalInput").ap(),
        "wq": nc.dram_tensor("wq", [C, 128], WDT, kind="ExternalInput").ap(),
        "wk": nc.dram_tensor("wk", [C, 128], WDT, kind="ExternalInput").ap(),
        "wv": nc.dram_tensor("wv", [C, 128], WDT, kind="ExternalInput").ap(),
        "bq": nc.dram_tensor("bq", [1, 128], WDT, kind="ExternalInput").ap(),
        "bk": nc.dram_tensor("bk", [1, 128], WDT, kind="ExternalInput").ap(),
        "bv": nc.dram_tensor("bv", [1, 128], WDT, kind="ExternalInput").ap(),
        "wp": nc.dram_tensor("wp", [128, C], WDT, kind="ExternalInput").ap(),
        "identh": nc.dram_tensor("identh", [128, 64], WDT, kind="ExternalInput").ap(),
        "mask_k": nc.dram_tensor("mask_k", [64, T], WDT, kind="ExternalInput").ap(),
        "mask_q": nc.dram_tensor("mask_q", [64, T], WDT, kind="ExternalInput").ap(),
        "sel65": nc.dram_tensor("sel65", [65, 1], WDT, kind="ExternalInput").ap(),
        "ones_row": nc.dram_tensor("ones_row", [1, T], WDT, kind="ExternalInput").ap(),
        "ones_col": nc.dram_tensor("ones_col", [128, 1], WDT, kind="ExternalInput").ap(),
    }
    out_d = nc.dram_tensor("out", [T, C], BF16, kind="ExternalOutput").ap()
    with tile.TileContext(nc) as tc:
        with ExitStack() as ctx:
            _emit(ctx, tc, out_d, in_d)
    nc.compile()
    _PROGRAM_CACHE[key] = nc
    return nc


def _consts():
    f = np.float32
    rr = np.arange(64)
    jj = np.arange(T)
    # mask_k[r, j] = 1 where r == 32*(jb%2) + (j%128)//4 (jb = j//128)
    mk = (rr[:, None] == 32 * ((jj[None, :] // 128) % 2) + (jj[None, :] % 128) // NA)
    mask_k = mk.astype(f)
    # mask_q[r, i] = NEG where (i%256)//4 < r
    mask_q = np.where((jj[None, :] % IC) // NA < rr[:, None], f(NEG), f(0.0)).astype(f)
    sel = np.zeros((65, 1), dtype=f)
    sel[64, 0] = 1.0
    identh = np.vstack([np.eye(64, dtype=f), np.eye(64, dtype=f)])
    return mask_k, mask_q, sel, identh


def _numpy_reference(x, mask, Wq, bq, Wk, bk, Wv, bv, Wp, bp):
    b, t, c = x.shape
    h, d = H, c // H
    scale = 1.0 / math.sqrt(d)
    tril = np.tril(np.ones((NF, NF), dtype=np.float32))
    block = np.kron(tril, np.ones((NA, NA), dtype=np.float32))

    def heads(w, bias):
        return (x @ w + bias).reshape(b, t, h, d).transpose(0, 2, 1, 3)

    q, k, v = heads(Wq, bq), heads(Wk, bk), heads(Wv, bv)
    att = np.einsum("bhqd,bhkd->bhqk", q, k) * scale
    allowed = block[None, None] * mask[:, None, None, :].astype(np.float32)
    att = np.where(allowed == 0, -np.inf, att)
    att = att - att.max(axis=-1, keepdims=True)
    att = np.exp(att)
    att = att / att.sum(axis=-1, keepdims=True)
    y = np.einsum("bhqk,bhkd->bhqd", att, v)
    y = y.transpose(0, 2, 1, 3).reshape(b, t, c)
    return (y @ Wp + bp).astype(np.float32)


def kernel(**inputs):
    x = np.asarray(inputs["x"], dtype=np.float32)
    mask = np.asarray(inputs["mask"])
    Wq = np.asarray(inputs["Wq"], dtype=np.float32)
    bq = np.asarray(inputs["bq"], dtype=np.float32)
    Wk = np.asarray(inputs["Wk"], dtype=np.float32)
    bk = np.asarray(inputs["bk"], dtype=np.float32)
    Wv = np.asarray(inputs["Wv"], dtype=np.float32)
    bv = np.asarray(inputs["bv"], dtype=np.float32)
    Wp = np.asarray(inputs["Wp"], dtype=np.float32)
    bp = np.asarray(inputs["bp"], dtype=np.float32)

    if not np.all(np.asarray(mask) == 1):
        return _numpy_reference(x, mask, Wq, bq, Wk, bk, Wv, bv, Wp, bp)

    nc = _build_program()
    mask_k, mask_q, sel, identh = _consts()
    bf = NPBF16
    in_maps = []
    for core in range(NCORES):
        b = core // CPB
        hp = core % CPB
        cs = slice(hp * 128, (hp + 1) * 128)
        in_maps.append({
            "xt": np.ascontiguousarray(x[b].T).astype(bf),
            "wq": (np.ascontiguousarray(Wq[:, cs]) * np.float32(SCALE)).astype(bf),
            "wk": np.ascontiguousarray(Wk[:, cs]).astype(bf),
            "wv": np.ascontiguousarray(Wv[:, cs]).astype(bf),
            "bq": (bq[cs] * np.float32(SCALE)).reshape(1, 128).astype(bf),
            "bk": bk[cs].reshape(1, 128).astype(bf),
            "bv": bv[cs].reshape(1, 128).astype(bf),
            "wp": np.ascontiguousarray(Wp[cs, :]).astype(bf),
            "identh": identh.astype(bf),
            "mask_k": mask_k.astype(bf),
            "mask_q": mask_q.astype(bf),
            "sel65": sel.astype(bf),
            "ones_row": np.ones((1, T), dtype=bf),
            "ones_col": np.ones((128, 1), dtype=bf),
        })
    rr = run_bass_kernel_spmd(
        nc, in_maps, list(range(NCORES)), trace=TRACE,
        tmpdir=_LAST.get("tmpdir"),
    )
    _LAST["results"] = rr
    res = rr.results
    out = np.zeros((B, T, C), dtype=np.float32)
    for b in range(B):
        acc = res[b * CPB]["out"].astype(np.float32)
        for j in range(1, CPB):
            acc = acc + res[b * CPB + j]["out"]
        out[b] = acc + bp[None, :]
    return out


# revision 12
# speedup vs baseline: 2.0007x; 1.3746x over previous
"""Block-causal multi-head self-attention for TRN2, sharded over 8 NeuronCores.

Problem (hardcoded): B=2, T=2048 (512 frames x 4 animals), C=512, H=8 heads,
D=64. Block-causal mask = kron(tril(frames), ones(4,4)); key mask is all-ones
per the input spec (a numpy fallback handles the general case).

Sharding: core i handles batch b = i//4 and heads (2*(i%4), 2*(i%4)+1).
Wq/Wk/Wv are column-sharded (128 cols per core), Wp row-sharded (128 rows per
core). Each core emits a bf16 partial [T, C] output projection; the host sums
the 4 partials per batch (fp32) and adds bp.

On-core algorithm (bf16 matmul data, fp32 PSUM accumulation), fully
software-pipelined: per 512-token chunk t5, the q/k/v projections and V-block
transposes are emitted, then the attention groups for query chunks 2*t5 and
2*t5+1 flow through an S -> exp -> P@V pipeline with 2-group lookahead so the
PE never waits on the scalar-engine exp. Mask handled by a rank-64 indicator
matmul on diagonal blocks (-1e9 offside). The softmax denominator l rides as
row 64 of the accumulated O^T (ones column in V_aug); division by l happens
after the output projection, per 128-row tile, split between ACT and DVE.
"""

import math

import numpy as np
import ml_dtypes

import concourse.bass as bass
import concourse.bacc as bacc
import concourse.tile as tile
from concourse import mybir
from concourse.bass_utils import run_bass_kernel_spmd

B, T, C, H, D = 2, 2048, 512, 8, 64
NF, NA = 512, 4
NCORES = 8
HPC = 2            # heads per core
CPB = 4            # cores per batch
SCALE = 1.0 / math.sqrt(D)
NEG = -1.0e9
IC = 256           # query-chunk width
NQC = T // IC      # 8
GRP = 4            # j-blocks (128 keys each) per exp group

F32 = mybir.dt.float32
BF16 = mybir.dt.bfloat16
NPBF16 = ml_dtypes.bfloat16

USE_BIAS = False     # set by kernel() when any bias is nonzero
DBG_NOMASK = False


def _emit(ctx, tc, out_d, in_d):
    nc = tc.nc
    WDT = BF16
    ts = bass.ts
    Exp = mybir.ActivationFunctionType.Exp
    mult, add = mybir.AluOpType.mult, mybir.AluOpType.add

    const = ctx.enter_context(tc.tile_pool(name="const", bufs=1))
    big = ctx.enter_context(tc.tile_pool(name="big", bufs=1))

    # --- SBUF tiles -------------------------------------------------------
    sel = const.tile([65, 1], WDT)
    ident = const.tile([128, 128], WDT)
    ones_col = const.tile([128, 1], WDT)
    w_sb = {nm: const.tile([128, 512], WDT, tag=f"w_{nm}", name=f"w_{nm}")
            for nm in ("wq", "wk", "wv")}
    wps = [const.tile([64, 512], WDT, tag=f"wp{h}", name=f"wp{h}")
           for h in range(HPC)]
    if USE_BIAS:
        b3 = const.tile([1, 384], WDT, tag="b3")
        ones_row = const.tile([1, T], WDT, tag="ones")

    xT = big.tile([128, 4 * T], WDT, tag="xT")
    qA = [big.tile([128, T], WDT, tag=f"qA{h}", name=f"qA{h}") for h in range(HPC)]
    kA = [big.tile([128, T], WDT, tag=f"kA{h}", name=f"kA{h}") for h in range(HPC)]
    vT = big.tile([128, T], WDT, tag="vT")
    Vb = big.tile([128, 16 * 130], WDT, tag="Vb")
    OTs = [big.tile([65, T], WDT, tag=f"OT{h}", name=f"OT{h}") for h in range(HPC)]
    rl = big.tile([128, 2 * 16], F32, tag="rl")

    # --- input DMAs, split across the two HWDGE rings ---------------------
    # scalar ring: masks (needed by first S groups), small consts, xt half 1
    nc.scalar.dma_start(out=kA[0][64:128, :], in_=in_d["mask_k"][:])
    nc.scalar.dma_start(out=kA[1][0:64, :], in_=in_d["mask_k"][:])
    nc.scalar.dma_start(out=qA[0][64:128, :], in_=in_d["mask_q"][:])
    nc.scalar.dma_start(out=qA[1][0:64, :], in_=in_d["mask_q"][:])
    nc.scalar.dma_start(out=sel[:], in_=in_d["sel65"][:])
    nc.scalar.dma_start(out=ident[:], in_=in_d["ident"][:])
    nc.scalar.dma_start(out=ones_col[:], in_=in_d["ones_col"][:])
    if USE_BIAS:
        nc.scalar.dma_start(out=b3[:], in_=in_d["b3"][:])
        nc.scalar.dma_start(out=ones_row[:], in_=in_d["ones_row"][:])
    # sync ring: weights + xt half 0; earliest-needed first
    for cb in range(4):
        nc.sync.dma_start(
            out=xT[:, cb * T : cb * T + 1024], in_=in_d["xt"][ts(cb, 128), 0:1024]
        )
    nc.sync.dma_start(out=w_sb["wq"][:], in_=in_d["wq"][:])
    nc.sync.dma_start(out=w_sb["wk"][:], in_=in_d["wk"][:])
    nc.sync.dma_start(out=w_sb["wv"][:], in_=in_d["wv"][:])
    nc.sync.dma_start(out=wps[0][:], in_=in_d["wp"][0:64, :])
    nc.sync.dma_start(out=wps[1][:], in_=in_d["wp"][64:128, :])
    for cb in range(4):
        nc.scalar.dma_start(
            out=xT[:, cb * T + 1024 : cb * T + 2048],
            in_=in_d["xt"][ts(cb, 128), 1024:2048],
        )

    # Vb ones columns (constant): cols jb*130+64 and jb*130+129.
    for jb in range(16):
        nc.gpsimd.tensor_copy(Vb[:, jb * 130 + 64 : jb * 130 + 65], ones_col[:])
        nc.gpsimd.tensor_copy(Vb[:, jb * 130 + 129 : jb * 130 + 130], ones_col[:])

    # --- attention group list --------------------------------------------
    flat = []
    for qc in range(NQC):
        nbj = 2 * (qc + 1)
        for h in range(HPC):
            for g0 in range(0, nbj, GRP):
                flat.append((qc, h, g0, min(GRP, nbj - g0), nbj))
    n = len(flat)
    gstart = {}  # t5 -> first group index of qc == 2*t5
    for i, (qc, hh, g0, ng, nbj) in enumerate(flat):
        gstart.setdefault(qc // 2, i)
    gend = {t5: gstart.get(t5 + 1, n) for t5 in range(4)}

    pss = ctx.enter_context(tc.tile_pool(name="pss", bufs=2, space="PSUM"))
    psot = ctx.enter_context(tc.tile_pool(name="psot", bufs=1, space="PSUM"))
    prl = ctx.enter_context(tc.tile_pool(name="prl", bufs=2, space="PSUM"))
    aux = ctx.enter_context(tc.tile_pool(name="aux", bufs=1, space="PSUM"))
    ptp = ctx.enter_context(tc.tile_pool(name="ptp", bufs=3))
    comb = ctx.enter_context(tc.tile_pool(name="comb", bufs=2))

    ps_t = {}
    ot_t = {}
    s_ptr = [0]

    def emit_s(i):
        qc, h, g0, ng, nbj = flat[i]
        ps = pss.tile([128, GRP * IC], F32, tag="ps", name=f"ps{i}")
        ps_t[i] = ps
        hs = slice(0, 64) if h == 0 else slice(64, 128)
        for k in range(ng):
            jb = g0 + k
            sl = ps[:, ts(k, IC)]
            if jb >= 2 * qc and not DBG_NOMASK:
                nc.tensor.matmul(
                    sl, kA[h][:, ts(jb, 128)], qA[h][:, ts(qc, IC)],
                    start=True, stop=True,
                )
            else:
                nc.tensor.matmul(
                    sl, kA[h][hs, ts(jb, 128)], qA[h][hs, ts(qc, IC)],
                    start=True, stop=True,
                )

    def pump_s(upto):
        while s_ptr[0] < min(upto, n):
            emit_s(s_ptr[0])
            s_ptr[0] += 1

    def emit_proj(nm, dst, t5):
        ps = aux.tile([128, 512], F32, tag="aux", name=f"pj_{nm}{t5}")
        for cb in range(4):
            nc.tensor.matmul(
                ps[:],
                w_sb[nm][:, ts(cb, 128)],
                xT[:, cb * T + t5 * 512 : cb * T + (t5 + 1) * 512],
                start=(cb == 0),
                stop=(False if USE_BIAS else cb == 3),
            )
        if USE_BIAS:
            boff = {"wq": 0, "wk": 128, "wv": 256}[nm]
            nc.tensor.matmul(
                ps[:], b3[:, boff : boff + 128], ones_row[:, ts(t5, 512)],
                start=False, stop=True,
            )
        if nm == "wv":
            nc.vector.tensor_copy(dst[:, ts(t5, 512)], ps[:])
        else:
            nc.vector.tensor_copy(dst[0][0:64, ts(t5, 512)], ps[0:64, :])
            nc.vector.tensor_copy(dst[1][64:128, ts(t5, 512)], ps[64:128, :])

    def emit_vb(t5):
        for jb in range(4 * t5, 4 * t5 + 4):
            pv = aux.tile([128, 128], WDT, tag="aux", name=f"pv{jb}")
            nc.tensor.transpose(pv[:], vT[:, ts(jb, 128)], ident[:])
            nc.vector.tensor_copy(Vb[:, jb * 130 : jb * 130 + 64], pv[:, 0:64])
            nc.vector.tensor_copy(
                Vb[:, jb * 130 + 65 : jb * 130 + 129], pv[:, 64:128]
            )

    def emit_tail(qc):
        for t2 in (2 * qc, 2 * qc + 1):
            pr1 = prl.tile([128, 512], F32, tag="pr", name=f"pr1_{t2}")
            for h in range(HPC):
                nc.tensor.matmul(
                    pr1[:, h : h + 1], OTs[h][:, ts(t2, 128)], sel[:],
                    start=True, stop=True,
                )
            nc.vector.reciprocal(rl[:, 2 * t2 : 2 * t2 + 2], pr1[:, 0:2])
            pr0 = prl.tile([128, 512], F32, tag="pr", name=f"pr0_{t2}")
            nc.tensor.matmul(
                pr0[:], OTs[0][0:64, ts(t2, 128)], wps[0][:],
                start=True, stop=True,
            )
            nc.tensor.matmul(
                pr1[:], OTs[1][0:64, ts(t2, 128)], wps[1][:],
                start=True, stop=True,
            )
            tmp = comb.tile([128, 512], F32, tag="tmp")
            if t2 % 2 == 0:
                nc.scalar.mul(tmp[:], pr0[:], rl[:, 2 * t2 : 2 * t2 + 1])
            else:
                nc.vector.tensor_scalar_mul(
                    tmp[:], pr0[:], rl[:, 2 * t2 : 2 * t2 + 1]
                )
            ob = comb.tile([128, 512], WDT, tag="ob")
            nc.vector.scalar_tensor_tensor(
                ob[:], pr1[:], rl[:, 2 * t2 + 1 : 2 * t2 + 2], tmp[:], mult, add
            )
            nc.sync.dma_start(out=out_d[ts(t2, 128), :], in_=ob[:])

    # --- main pipeline ----------------------------------------------------
    for t5 in range(4):
        emit_proj("wq", qA, t5)
        emit_proj("wk", kA, t5)
        pump_s(gstart[t5] + 2)
        emit_proj("wv", vT, t5)
        emit_vb(t5)
        for i in range(gstart[t5], gend[t5]):
            qc, h, g0, ng, nbj = flat[i]
            ps = ps_t.pop(i)
            ptt = ptp.tile([128, GRP * IC], WDT)
            nc.scalar.activation(ptt[:, 0 : ng * IC], ps[:, 0 : ng * IC], Exp)
            if g0 == 0 and h == 0:
                ot_t[qc] = psot.tile([65, HPC * IC], F32, tag="ot", name=f"ot{qc}")
            ot = ot_t[qc]
            osl = ot[:, h * IC : (h + 1) * IC]
            for k in range(ng):
                jb = g0 + k
                vbase = jb * 130 + h * 65
                nc.tensor.matmul(
                    osl, Vb[:, vbase : vbase + 65], ptt[:, ts(k, IC)],
                    start=(jb == 0), stop=(jb == nbj - 1),
                )
            pump_s(min(i + 3, gend[t5]))
            if g0 + ng == nbj:
                nc.vector.tensor_copy(OTs[h][:, ts(qc, IC)], osl)
                if h == HPC - 1:
                    emit_tail(qc)


_PROGRAM_CACHE = {}
TRACE = False
_LAST = {}


def _build_program():
    key = ("prog", GRP, USE_BIAS, DBG_NOMASK)
    if key in _PROGRAM_CACHE:
        return _PROGRAM_CACHE[key]
    from contextlib import ExitStack

    nc = bacc.Bacc(trn_type="TRN2", target_bir_lowering=False, debug=False,
                   num_devices=NCORES)
    WDT = BF16
    in_d = {
        "xt": nc.dram_tensor("xt", [C, T], WDT, kind="ExternalInput").ap(),
        "wq": nc.dram_tensor("wq", [128, 512], WDT, kind="ExternalInput").ap(),
        "wk": nc.dram_tensor("wk", [128, 512], WDT, kind="ExternalInput").ap(),
        "wv": nc.dram_tensor("wv", [128, 512], WDT, kind="ExternalInput").ap(),
        "wp": nc.dram_tensor("wp", [128, C], WDT, kind="ExternalInput").ap(),
        "ident": nc.dram_tensor("ident", [128, 128], WDT, kind="ExternalInput").ap(),
        "mask_k": nc.dram_tensor("mask_k", [64, T], WDT, kind="ExternalInput").ap(),
        "mask_q": nc.dram_tensor("mask_q", [64, T], WDT, kind="ExternalInput").ap(),
        "sel65": nc.dram_tensor("sel65", [65, 1], WDT, kind="ExternalInput").ap(),
        "ones_col": nc.dram_tensor("ones_col", [128, 1], WDT,
                                   kind="ExternalInput").ap(),
    }
    if USE_BIAS:
        in_d["b3"] = nc.dram_tensor("b3", [1, 384], WDT, kind="ExternalInput").ap()
        in_d["ones_row"] = nc.dram_tensor(
            "ones_row", [1, T], WDT, kind="ExternalInput").ap()
    out_d = nc.dram_tensor("out", [T, C], BF16, kind="ExternalOutput").ap()
    with tile.TileContext(nc) as tc:
        with ExitStack() as ctx:
            _emit(ctx, tc, out_d, in_d)
    nc.compile()
    _PROGRAM_CACHE[key] = nc
    return nc


def _consts():
    f = np.float32
    rr = np.arange(64)
    jj = np.arange(T)
    # mask_k[r, j] = 1 where r == 32*(jb%2) + (j%128)//4 (jb = j//128)
    mk = (rr[:, None] == 32 * ((jj[None, :] // 128) % 2) + (jj[None, :] % 128) // NA)
    mask_k = mk.astype(f)
    # mask_q[r, i] = NEG where (i%256)//4 < r
    mask_q = np.where((jj[None, :] % IC) // NA < rr[:, None], f(NEG), f(0.0)).astype(f)
    sel = np.zeros((65, 1), dtype=f)
    sel[64, 0] = 1.0
    ident = np.eye(128, dtype=f)
    return mask_k, mask_q, sel, ident


def _sbuf_w(w):
    # DRAM [512, 128] -> SBUF-layout [128, 512]: w_sb[p, cb*128+f] = W[cb*128+p, f]
    return np.ascontiguousarray(
        w.reshape(4, 128, 128).transpose(1, 0, 2).reshape(128, 512)
    )


def _numpy_reference(x, mask, Wq, bq, Wk, bk, Wv, bv, Wp, bp):
    b, t, c = x.shape
    h, d = H, c // H
    scale = 1.0 / math.sqrt(d)
    tril = np.tril(np.ones((NF, NF), dtype=np.float32))
    block = np.kron(tril, np.ones((NA, NA), dtype=np.float32))

    def heads(w, bias):
        return (x @ w + bias).reshape(b, t, h, d).transpose(0, 2, 1, 3)

    q, k, v = heads(Wq, bq), heads(Wk, bk), heads(Wv, bv)
    att = np.einsum("bhqd,bhkd->bhqk", q, k) * scale
    allowed = block[None, None] * mask[:, None, None, :].astype(np.float32)
    att = np.where(allowed == 0, -np.inf, att)
    att = att - att.max(axis=-1, keepdims=True)
    att = np.exp(att)
    att = att / att.sum(axis=-1, keepdims=True)
    y = np.einsum("bhqk,bhkd->bhqd", att, v)
    y = y.transpose(0, 2, 1, 3).reshape(b, t, c)
    return (y @ Wp + bp).astype(np.float32)


def kernel(**inputs):
    global USE_BIAS
    x = np.asarray(inputs["x"], dtype=np.float32)
    mask = np.asarray(inputs["mask"])
    Wq = np.asarray(inputs["Wq"], dtype=np.float32)
    bq = np.asarray(inputs["bq"], dtype=np.float32)
    Wk = np.asarray(inputs["Wk"], dtype=np.float32)
    bk = np.asarray(inputs["bk"], dtype=np.float32)
    Wv = np.asarray(inputs["Wv"], dtype=np.float32)
    bv = np.asarray(inputs["bv"], dtype=np.float32)
    Wp = np.asarray(inputs["Wp"], dtype=np.float32)
    bp = np.asarray(inputs["bp"], dtype=np.float32)

    if not np.all(np.asarray(mask) == 1):
        return _numpy_reference(x, mask, Wq, bq, Wk, bk, Wv, bv, Wp, bp)

    USE_BIAS = bool(np.any(bq) or np.any(bk) or np.any(bv))
    nc = _build_program()
    mask_k, mask_q, sel, ident = _consts()
    bf = NPBF16
    in_maps = []
    for core in range(NCORES):
        b = core // CPB
        hp = core % CPB
        cs = slice(hp * 128, (hp + 1) * 128)
        im = {
            "xt": np.ascontiguousarray(x[b].T).astype(bf),
            "wq": (_sbuf_w(np.ascontiguousarray(Wq[:, cs]))
                   * np.float32(SCALE)).astype(bf),
            "wk": _sbuf_w(np.ascontiguousarray(Wk[:, cs])).astype(bf),
            "wv": _sbuf_w(np.ascontiguousarray(Wv[:, cs])).astype(bf),
            "wp": np.ascontiguousarray(Wp[cs, :]).astype(bf),
            "ident": ident.astype(bf),
            "mask_k": mask_k.astype(bf),
            "mask_q": mask_q.astype(bf),
            "sel65": sel.astype(bf),
            "ones_col": np.ones((128, 1), dtype=bf),
        }
        if USE_BIAS:
            b3 = np.concatenate([bq[cs] * np.float32(SCALE), bk[cs], bv[cs]])
            im["b3"] = b3.reshape(1, 384).astype(bf)
            im["ones_row"] = np.ones((1, T), dtype=bf)
        in_maps.append(im)
    rr = run_bass_kernel_spmd(
        nc, in_maps, list(range(NCORES)), trace=TRACE,
        tmpdir=_LAST.get("tmpdir"),
    )
    _LAST["results"] = rr
    res = rr.results
    out = np.zeros((B, T, C), dtype=np.float32)
    for b in range(B):
        acc = res[b * CPB]["out"].astype(np.float32)
        for j in range(1, CPB):
            acc = acc + res[b * CPB + j]["out"].astype(np.float32)
        out[b] = acc + bp[None, :]
    return out
